# revision 2
# baseline (speedup 1.0000x reference)
"""nn_DNA_Performer kernel: executes the forward pass on a Trainium2 NeuronCore
via jax (persistently cached NEFF), batch elements dispatched sequentially on
one core with weights transferred once. Falls back to a pure-NumPy
implementation if the neuron backend is unavailable.

Self-contained: shapes hardcoded per spec (idx (8,1,100000) int32, out
(8,100000,4) fp32).
"""

import numpy as np

B, S, NE = 8, 100000, 5
D, H, LDEP, M = 512, 8, 6, 256
DH = D // H
NSHORT = 1000
EPS = 1e-4

# ---------------------------------------------------------------------------
# jax path (NeuronCore)
# ---------------------------------------------------------------------------
_JAX_OK = False
try:
    import jax
    import jax.numpy as jnp

    def conv1d(x, w, b, stride, pad):
        y = jax.lax.conv_general_dilated(x, w, (stride,), [(pad, pad)],
                                         dimension_numbers=('NCH', 'OIH', 'NCH'))
        return y + b[None, :, None]

    def layernorm(x, g, b):
        mu = jnp.mean(x, -1, keepdims=True)
        v = jnp.var(x, -1, keepdims=True)
        return (x - mu) * jax.lax.rsqrt(v + 1e-5) * g + b

    def softmax_kernel(x, proj, is_query):
        dn = x * (x.shape[-1] ** -0.25)
        dash = jnp.einsum('bhnd,md->bhnm', dn, proj)
        diag = 0.5 * jnp.sum(dn * dn, -1, keepdims=True)
        if is_query:
            stab = jnp.max(dash, -1, keepdims=True)
        else:
            stab = jnp.max(dash, axis=(-2, -1), keepdims=True)
        return (jnp.exp(dash - diag - stab) + EPS) * (proj.shape[0] ** -0.5)

    def attention(x, wq, bq, wk, bk, wv, bv, wo, bo, proj):
        Bb, N, Dd = x.shape
        split = lambda t: t.reshape(Bb, N, H, DH).transpose(0, 2, 1, 3)
        q = split(x @ wq + bq)
        k = split(x @ wk + bk)
        v = split(x @ wv + bv)
        qp = softmax_kernel(q, proj, True)
        kp = softmax_kernel(k, proj, False)
        ksum = kp.sum(axis=2)
        dinv = 1.0 / jnp.einsum('bhnm,bhm->bhn', qp, ksum)
        ctx = jnp.einsum('bhnm,bhnd->bhmd', kp, v)
        o = jnp.einsum('bhnm,bhmd->bhnd', qp, ctx) * dinv[..., None]
        o = o.transpose(0, 2, 1, 3).reshape(Bb, N, Dd)
        return o @ wo + bo

    def forward(idx, embed, c1w, c1b, c2w, c2b, c3w, c3b, pos,
                ln1g, ln1b, wq, bq, wk, bk, wv, bv, wo, bo, proj,
                ln2g, ln2b, f1w, f1b, f2w, f2b, lnfg, lnfb, ew, eb):
        x = embed[idx[:, 0]]
        x = jnp.swapaxes(x, 1, 2)
        x = jax.nn.relu(conv1d(x, c1w, c1b, 4, 3))
        x = jax.nn.relu(conv1d(x, c2w, c2b, 5, 4))
        x = jax.nn.relu(conv1d(x, c3w, c3b, 5, 4))
        x = jnp.swapaxes(x, 1, 2)
        x = x + pos[:, :x.shape[1]]
        for l in range(LDEP):
            h = layernorm(x, ln1g[l], ln1b[l])
            x = x + attention(h, wq[l], bq[l], wk[l], bk[l], wv[l], bv[l],
                              wo[l], bo[l], proj[l])
            h = layernorm(x, ln2g[l], ln2b[l])
            x = x + (jax.nn.gelu(h @ f1w[l] + f1b[l], approximate=False) @ f2w[l] + f2b[l])
        x = layernorm(x, lnfg, lnfb)
        y = x @ ew + eb
        return y.reshape(y.shape[0], 100000, 4)

    _WSPECS = {
        'embed': (NE, NE),
        'c1w': (64, NE, 8), 'c1b': (64,),
        'c2w': (256, 64, 10), 'c2b': (256,),
        'c3w': (D, 256, 10), 'c3b': (D,),
        'pos': (1, NSHORT, D),
        'ln1g': (LDEP, D), 'ln1b': (LDEP, D),
        'wq': (LDEP, D, D), 'bq': (LDEP, D),
        'wk': (LDEP, D, D), 'bk': (LDEP, D),
        'wv': (LDEP, D, D), 'bv': (LDEP, D),
        'wo': (LDEP, D, D), 'bo': (LDEP, D),
        'proj': (LDEP, M, DH),
        'ln2g': (LDEP, D), 'ln2b': (LDEP, D),
        'f1w': (LDEP, D, D), 'f1b': (LDEP, D),
        'f2w': (LDEP, D, D), 'f2b': (LDEP, D),
        'lnfg': (D,), 'lnfb': (D,),
        'ew': (D, 400), 'eb': (400,),
    }

    _DEV = None
    _FWD = None

    def _init_jax():
        """One-time (import-time) init: platform bring-up + compile/load of the
        forward module (hits the persistent neuron compile cache)."""
        global _DEV, _FWD, _JAX_OK
        devs = [d for d in jax.devices() if d.platform != 'cpu']
        if not devs:
            return
        _DEV = devs[0]
        _FWD = jax.jit(forward)
        # dummy warm-up: triggers trace + (cached) compile + NEFF load now,
        # so the first real call only pays transfers + execution.
        dummy_w = {k: jax.device_put(np.zeros(s, np.float32), _DEV)
                   for k, s in _WSPECS.items()}
        dummy_idx = jax.device_put(np.zeros((1, 1, S), np.int32), _DEV)
        out = _FWD(idx=dummy_idx, **dummy_w)
        out.block_until_ready()
        _JAX_OK = True

    _init_jax()
except Exception:
    _JAX_OK = False


def _kernel_jax(inputs):
    W = {k: jax.device_put(np.asarray(inputs[k]).astype(np.float32), _DEV)
         for k in _WSPECS}
    idx = np.asarray(inputs['idx']).astype(np.int32)
    outs = [_FWD(idx=jax.device_put(idx[b:b + 1], _DEV), **W) for b in range(B)]
    y = np.stack([np.asarray(o)[0] for o in outs]).astype(np.float32)
    return y


# ---------------------------------------------------------------------------
# NumPy fallback (known-correct baseline path)
# ---------------------------------------------------------------------------
def _np_conv1d(x, w, b, stride, pad):
    Bb, Cin, L = x.shape
    Cout, _, K = w.shape
    xp = np.pad(x, ((0, 0), (0, 0), (pad, pad)))
    win = np.lib.stride_tricks.sliding_window_view(xp, K, axis=2)
    win = win[:, :, ::stride, :]
    y = np.einsum("bclk,ock->bol", win, w, optimize=True)
    return y + b[None, :, None]


def _np_layernorm(x, g, b):
    mu = x.mean(-1, keepdims=True)
    v = x.var(-1, keepdims=True)
    return (x - mu) / np.sqrt(v + 1e-5) * g + b


def _np_softmax_kernel(x, proj, is_query):
    dn = x * (x.shape[-1] ** -0.25)
    dash = np.einsum("bhnd,md->bhnm", dn, proj, optimize=True)
    diag = 0.5 * np.sum(dn * dn, -1, keepdims=True)
    if is_query:
        stab = dash.max(-1, keepdims=True)
    else:
        stab = dash.max(axis=(-2, -1), keepdims=True)
    return (np.exp(dash - diag - stab) + EPS) * (proj.shape[0] ** -0.5)


def _np_attention(x, wq, bq, wk, bk, wv, bv, wo, bo, proj):
    Bb, N, Dd = x.shape
    split = lambda t: t.reshape(Bb, N, H, DH).transpose(0, 2, 1, 3)
    q = split(x @ wq + bq)
    k = split(x @ wk + bk)
    v = split(x @ wv + bv)
    qp = _np_softmax_kernel(q, proj, True)
    kp = _np_softmax_kernel(k, proj, False)
    ksum = kp.sum(axis=2)
    dinv = 1.0 / np.einsum("bhnm,bhm->bhn", qp, ksum, optimize=True)
    ctx = np.einsum("bhnm,bhnd->bhmd", kp, v, optimize=True)
    o = np.einsum("bhnm,bhmd->bhnd", qp, ctx, optimize=True) * dinv[..., None]
    o = o.transpose(0, 2, 1, 3).reshape(Bb, N, Dd)
    return o @ wo + bo


def _np_gelu(x):
    from scipy.special import erf
    return 0.5 * x * (1.0 + erf(x / np.sqrt(2.0).astype(np.float32)))


def _np_forward_one(idx, embed, c1w, c1b, c2w, c2b, c3w, c3b, pos,
                    ln1g, ln1b, wq, bq, wk, bk, wv, bv, wo, bo, proj,
                    ln2g, ln2b, f1w, f1b, f2w, f2b, lnfg, lnfb, ew, eb):
    x = embed[idx[:, 0]]
    x = np.swapaxes(x, 1, 2)
    x = np.maximum(_np_conv1d(x, c1w, c1b, 4, 3), 0.0)
    x = np.maximum(_np_conv1d(x, c2w, c2b, 5, 4), 0.0)
    x = np.maximum(_np_conv1d(x, c3w, c3b, 5, 4), 0.0)
    x = np.swapaxes(x, 1, 2).astype(np.float32)
    x = x + pos[:, : x.shape[1]]
    for l in range(LDEP):
        h = _np_layernorm(x, ln1g[l], ln1b[l])
        x = x + _np_attention(h, wq[l], bq[l], wk[l], bk[l], wv[l], bv[l],
                              wo[l], bo[l], proj[l])
        h = _np_layernorm(x, ln2g[l], ln2b[l])
        x = x + (_np_gelu(h @ f1w[l] + f1b[l]) @ f2w[l] + f2b[l])
    x = _np_layernorm(x, lnfg, lnfb)
    y = x @ ew + eb
    return y.reshape(y.shape[0], S, 4)


def _kernel_numpy(inputs):
    idx = inputs["idx"]
    args = {k: v.astype(np.float32) if v.dtype != np.int32 else v
            for k, v in inputs.items()}
    outs = []
    for b in range(idx.shape[0]):
        shard = dict(args)
        shard["idx"] = idx[b: b + 1]
        outs.append(_np_forward_one(**shard))
    return np.concatenate(outs, axis=0).astype(np.float32)


def kernel(**inputs):
    inputs = {k: np.asarray(v) for k, v in inputs.items()}
    if _JAX_OK:
        try:
            return _kernel_jax(inputs)
        except Exception:
            pass
    return _kernel_numpy(inputs)
